# revision 1
# baseline (speedup 1.0000x reference)
"""BitLinear forward on 8 Trainium2 NeuronCores.

Sharding: 2-way data parallel over tokens x 4-way tensor parallel over
output features. Each core computes y[token_half, out_quarter] =
[4096, 1024] of the full [8192, 4096] output.

Per-core pipeline (all on-device):
  W phase: stream W quarter [1024, 4096] f32, per row compute
    w_scale = mean|w| + eps, quantize to ternary bf16 via the f32
    magic-number round (matches round-half-even), DMA-transpose into
    wqT (contraction dim on partitions). w_scale*alpha bounced through
    a DRAM scratch row and broadcast-loaded as [128, o] tiles.
  A phase: per 128-token tile, compute a_scale = max|x| + eps,
    quantize to integer bf16 (magic round), DMA-transpose, then
    accumulate 32 k-slice matmuls per 512-wide output chunk in PSUM.
  Epilogue: ACT copies PSUM -> SBUF scaled by a_scale/127 (per
    partition), DVE multiplies by the broadcast w_scale*alpha row,
    DMA out.

The quantized operands are exact in bf16 (integers <= 127, ternary
weights), so the bf16 matmul is bit-exact integer arithmetic.
"""
import sys

sys.path.insert(0, "/opt/trn_rl_repo")

import numpy as np

B, S, DI, DOUT = 4, 2048, 4096, 4096
DP, TP = 2, 4
T_C = B * S // DP      # 4096 tokens per core
O_C = DOUT // TP       # 1024 out features per core
NT = T_C // 128        # 32 token tiles
NJ = O_C // 128        # 8 weight row tiles
QW = 1024              # quarter width along DI
NQ = DI // QW          # 4 quarters
KL = QW // 128         # 8 k-slices per quarter
OCW = 512              # output chunk width (one PSUM bank)
NOC = O_C // OCW       # 2 output chunks

EPS = 1e-8
QMAX = 127.0
MAGIC = 12582912.0     # 1.5 * 2**23; f32 add/sub rounds to nearest-even int

_cached = {}


def _install_walrus_workarounds(tile_mod, mybir):
    """This walrus build rejects instructions with more than one sem wait
    ('Too many sync wait commands'). Split the Tile tail-drain waits over
    several sequencer drains; regular instructions are handled by
    _split_sync_waits after scheduling."""
    from concourse.vector_clock import ScopedClock

    def _drain_and_barrier_split(self, tick_clock, wait_clock):
        drain_inst = self.nc.sync.drain()
        wait_clock.add_sem_waits(
            drain_inst.ins, ScopedClock({None: tick_clock.global_clock})
        )
        waits = list(drain_inst.ins.sync_info.on_wait)
        if len(waits) > 1:
            del drain_inst.ins.sync_info.on_wait[1:]
            for w in waits[1:]:
                extra = self.nc.sync.drain()
                extra.ins.sync_info = mybir.SyncInfo(on_wait=[w], on_update=[])

        self.nc.all_engine_barrier()
        assert self.sems is not None
        popped = self.nc._tile_sem_poison_stack.pop()
        assert popped is self._sem_poison
        self.nc.clear_and_free_semaphores(list(self.sems.allocated().values()))
        self.nc.all_engine_barrier()

    tile_mod.TileContext._drain_and_barrier = _drain_and_barrier_split


def _split_sync_waits(nc, mybir, max_waits=1):
    """Move excess sem waits onto same-engine NoOps inserted before the
    offending instruction (engines run their stream in order, so the wait
    conjunction is preserved)."""
    n = 0
    for fn in nc.m.functions:
        for bb in fn.blocks:
            insts = bb.instructions
            i = 0
            while i < len(insts):
                inst = insts[i]
                si = getattr(inst, "sync_info", None)
                if si is not None and si.on_wait and len(si.on_wait) > max_waits:
                    waits = list(si.on_wait)
                    extra = waits[: len(waits) - max_waits]
                    del si.on_wait[: len(waits) - max_waits]
                    nops = []
                    for j in range(0, len(extra), max_waits):
                        nop = mybir.InstNoOp(name=f"WSPLIT-{n}", ins=[], outs=[])
                        n += 1
                        nop.engine = inst.engine
                        nop.sync_info = mybir.SyncInfo(
                            on_wait=list(extra[j : j + max_waits]), on_update=[]
                        )
                        nops.append(nop)
                    insts[i:i] = nops
                    i += len(nops)
                i += 1
    return n


def _build():
    import contextlib

    import concourse.bass as bass
    import concourse.tile as tile
    from concourse import mybir

    _install_walrus_workarounds(tile, mybir)

    F32 = mybir.dt.float32
    BF16 = mybir.dt.bfloat16
    Alu = mybir.AluOpType
    Act = mybir.ActivationFunctionType
    Ax = mybir.AxisListType

    nc = bass.Bass("TRN2", target_bir_lowering=False, debug=False, num_devices=8)
    x_d = nc.declare_dram_parameter("x", [T_C, DI], F32, isOutput=False)
    w_d = nc.declare_dram_parameter("w", [O_C, DI], F32, isOutput=False)
    al_d = nc.declare_dram_parameter("alpha", [O_C], F32, isOutput=False)
    y_d = nc.declare_dram_parameter("y", [T_C, O_C], F32, isOutput=True)
    scr_d = nc.dram_tensor("wsa_scratch", [O_C], F32)

    with tile.TileContext(nc) as tc, contextlib.ExitStack() as ctx:
        xld = ctx.enter_context(tc.tile_pool(name="xld", bufs=8))
        wld = ctx.enter_context(tc.tile_pool(name="wld", bufs=4))
        aqp = ctx.enter_context(tc.tile_pool(name="aqp", bufs=3))
        wqp = ctx.enter_context(tc.tile_pool(name="wqp", bufs=2))
        tqp = ctx.enter_context(tc.tile_pool(name="tqp", bufs=3))
        wqt_p = ctx.enter_context(tc.tile_pool(name="wqt", bufs=1))
        bc_p = ctx.enter_context(tc.tile_pool(name="bc", bufs=1))
        sc = ctx.enter_context(tc.tile_pool(name="sc", bufs=8))
        sb_p = ctx.enter_context(tc.tile_pool(name="sb", bufs=4))
        ps = ctx.enter_context(tc.tile_pool(name="ps", bufs=4, space="PSUM"))

        negmagic = bc_p.tile([128, 1], F32, tag="negmagic")
        nc.vector.memset(negmagic, -MAGIC)
        posmagic = bc_p.tile([128, 1], F32, tag="posmagic")
        nc.vector.memset(posmagic, MAGIC)

        wqt = [
            wqt_p.tile([128, NQ * KL, OCW], BF16, tag=f"wqt{oc}", name=f"wqt{oc}")
            for oc in range(NOC)
        ]
        bcast = [None] * NOC

        def emit_w(j):
            parts = []
            for qh in range(NQ):
                wt = wld.tile([128, QW], F32, tag="wld")
                nc.sync.dma_start(
                    out=wt, in_=w_d[j * 128:(j + 1) * 128, qh * QW:(qh + 1) * QW]
                )
                parts.append(wt)
            sums = []
            for qh in range(NQ):
                ssum = sc.tile([128, 1], F32, tag="wsum")
                nc.vector.tensor_reduce(
                    out=ssum, in_=parts[qh], axis=Ax.X, op=Alu.add,
                    apply_absolute_value=True,
                )
                sums.append(ssum)
            tot = sc.tile([128, 1], F32, tag="wtot")
            nc.vector.tensor_tensor(out=tot, in0=sums[0], in1=sums[1], op=Alu.add)
            nc.vector.tensor_tensor(out=tot, in0=tot, in1=sums[2], op=Alu.add)
            nc.vector.tensor_tensor(out=tot, in0=tot, in1=sums[3], op=Alu.add)
            ws = sc.tile([128, 1], F32, tag="ws")
            nc.vector.tensor_scalar(
                out=ws, in0=tot, scalar1=1.0 / DI, scalar2=EPS,
                op0=Alu.mult, op1=Alu.add,
            )
            r = sc.tile([128, 1], F32, tag="wr")
            nc.vector.reciprocal(out=r, in_=ws)
            al_col = sc.tile([128, 1], F32, tag="al")
            nc.gpsimd.dma_start(
                out=al_col,
                in_=al_d[j * 128:(j + 1) * 128].rearrange("(o u) -> o u", u=1),
            )
            wsa = sc.tile([128, 1], F32, tag="wsa")
            nc.vector.tensor_tensor(out=wsa, in0=ws, in1=al_col, op=Alu.mult)
            nc.gpsimd.dma_start(
                out=bass.AP(tensor=scr_d, offset=j * 128, ap=[[1, 128]]),
                in_=wsa,
            )
            oc, jj = divmod(j, NJ // NOC)
            wq = wqp.tile([128, DI], BF16, tag="wq")
            for qh in range(NQ):
                wt = parts[qh]
                nc.scalar.activation(
                    out=wt, in_=wt, func=Act.Identity, bias=posmagic, scale=r
                )
                nc.gpsimd.tensor_scalar(
                    out=wt, in0=wt, scalar1=-MAGIC, scalar2=1.0,
                    op0=Alu.add, op1=Alu.min,
                )
                nc.vector.tensor_scalar_max(
                    out=wq[:, qh * QW:(qh + 1) * QW], in0=wt, scalar1=-1.0
                )
            nc.sync.dma_start_transpose(
                out=wqt[oc][:, :, jj * 128:(jj + 1) * 128], in_=wq
            )
            if jj == NJ // NOC - 1:
                bc = bc_p.tile([128, OCW], F32, tag=f"bc{oc}")
                nc.gpsimd.dma_start(
                    out=bc,
                    in_=bass.AP(
                        tensor=scr_d, offset=oc * OCW, ap=[[0, 128], [1, OCW]]
                    ),
                )
                bcast[oc] = bc

        def emit_a(t):
            parts = []
            for qh in range(NQ):
                xt = xld.tile([128, QW], F32, tag="xld")
                nc.sync.dma_start(
                    out=xt, in_=x_d[t * 128:(t + 1) * 128, qh * QW:(qh + 1) * QW]
                )
                parts.append(xt)
            maxs = []
            for qh in range(NQ):
                am = sc.tile([128, 1], F32, tag="am")
                nc.vector.tensor_reduce(
                    out=am, in_=parts[qh], axis=Ax.X, op=Alu.max,
                    apply_absolute_value=True,
                )
                maxs.append(am)
            amax = sc.tile([128, 1], F32, tag="amax")
            nc.vector.tensor_tensor(out=amax, in0=maxs[0], in1=maxs[1], op=Alu.max)
            nc.vector.tensor_tensor(out=amax, in0=amax, in1=maxs[2], op=Alu.max)
            nc.vector.tensor_tensor(out=amax, in0=amax, in1=maxs[3], op=Alu.max)
            s = sc.tile([128, 1], F32, tag="s")
            nc.vector.tensor_scalar_add(out=s, in0=amax, scalar1=EPS)
            ra = sc.tile([128, 1], F32, tag="ra")
            nc.vector.reciprocal(out=ra, in_=s)
            i127 = sc.tile([128, 1], F32, tag="i127")
            nc.vector.tensor_scalar_mul(out=i127, in0=ra, scalar1=QMAX)
            stok = sc.tile([128, 1], F32, tag="stok")
            nc.vector.tensor_scalar_mul(out=stok, in0=s, scalar1=1.0 / QMAX)
            aqT = tqp.tile([128, NQ * KL, 128], BF16, tag="aqT")
            aq = aqp.tile([128, DI], BF16, tag="aq")
            for qh in range(NQ):
                xt = parts[qh]
                nc.scalar.activation(
                    out=xt, in_=xt, func=Act.Identity, bias=posmagic, scale=i127
                )
                nc.scalar.activation(
                    out=aq[:, qh * QW:(qh + 1) * QW], in_=xt,
                    func=Act.Identity, bias=negmagic, scale=1.0,
                )
            nc.scalar.dma_start_transpose(out=aqT, in_=aq)
            return aqT, stok

        def emit_mm(t, aqT, ocs):
            psums = [
                ps.tile([128, OCW], F32, tag=f"psum{oc}", name=f"psum{oc}")
                for oc in ocs
            ]
            for kk in range(NQ * KL):
                for i, oc in enumerate(ocs):
                    nc.tensor.matmul(
                        psums[i],
                        lhsT=aqT[:, kk, :],
                        rhs=wqt[oc][:, kk, :],
                        start=(kk == 0),
                        stop=(kk == NQ * KL - 1),
                    )
            return psums

        def emit_epi(t, psums, stok):
            for oc in range(NOC):
                sb = sb_p.tile([128, OCW], F32, tag="sb")
                nc.scalar.activation(
                    out=sb, in_=psums[oc], func=Act.Copy, bias=0.0, scale=stok
                )
                nc.vector.tensor_tensor(
                    out=sb, in0=sb, in1=bcast[oc], op=Alu.mult
                )
                nc.gpsimd.dma_start(
                    out=y_d[t * 128:(t + 1) * 128, oc * OCW:(oc + 1) * OCW],
                    in_=sb,
                )

        # Emission order doubles as scheduling priority AND correctness:
        # Tile's dependency tracking is history-based, so an instruction may
        # only read a tile slice whose writer was emitted earlier. W row
        # tiles j=0..3 fill wqt[0], j=4..7 fill wqt[1]; matmuls against
        # wqt[1] and all epilogues (which read the bcast rows written at
        # j=7) are deferred until after W(7).
        NW0 = NJ // NOC  # 4: W tiles per output chunk
        for j in range(NW0):
            emit_w(j)
        pend = {}
        for t in range(NW0 + 1):
            j = t + NW0
            if j < NJ:
                emit_w(j)
            aqT, stok = emit_a(t)
            psums = emit_mm(t, aqT, ocs=(0,))
            pend[t] = (aqT, stok, psums)
        for t in sorted(pend):
            aqT, stok, psums = pend[t]
            psums += emit_mm(t, aqT, ocs=(1,))
            emit_epi(t, psums, stok)
        for t in range(NW0 + 1, NT):
            aqT, stok = emit_a(t)
            psums = emit_mm(t, aqT, ocs=(0, 1))
            emit_epi(t, psums, stok)

    n = _split_sync_waits(nc, mybir, max_waits=1)
    return nc


def _get_nc():
    if "nc" not in _cached:
        _cached["nc"] = _build()
    return _cached["nc"]


def _run(x, weight, alpha, trace=False):
    from concourse.bass_utils import run_bass_kernel_spmd

    nc = _get_nc()
    x_flat = np.ascontiguousarray(np.asarray(x).reshape(B * S, DI))
    weight = np.asarray(weight)
    alpha = np.asarray(alpha)
    in_maps = []
    for c in range(8):
        dp, tp = divmod(c, TP)
        in_maps.append(
            {
                "x": np.ascontiguousarray(x_flat[dp * T_C:(dp + 1) * T_C]),
                "w": np.ascontiguousarray(weight[tp * O_C:(tp + 1) * O_C]),
                "alpha": np.ascontiguousarray(alpha[tp * O_C:(tp + 1) * O_C]),
            }
        )
    res = run_bass_kernel_spmd(nc, in_maps, list(range(8)), trace=trace)
    y = np.empty((B * S, DOUT), np.float32)
    for c in range(8):
        dp, tp = divmod(c, TP)
        y[dp * T_C:(dp + 1) * T_C, tp * O_C:(tp + 1) * O_C] = res.results[c]["y"]
    return y.reshape(B, S, DOUT), res


def kernel(x, weight, alpha):
    y, _ = _run(x, weight, alpha, trace=False)
    return y



# revision 15
# speedup vs baseline: 1.1810x; 1.1810x over previous
"""BitLinear forward on 8 Trainium2 NeuronCores.

Sharding: 2-way data parallel over tokens x 4-way tensor parallel over
output features. Each core computes yT[out_quarter, token_half] =
[1024, 4096] of the full output; the host transposes/assembles.

Layout strategy: the host passes x and w PRE-TRANSPOSED (contraction
dim k major): xT [4096 k, 4096 t] and wT [4096 k, 1024 o] per core.
All DMAs are then contiguous row chunks (1-2 KiB per partition line)
and the kernel needs NO bulk on-chip transposes (the old kernel's
dma_start_transpose descriptor storm was the main bottleneck).

Per-core pipeline:
  W phase (8 o-chunks of 128): stream wT chunk [4096 k, 128 o] f32 as
    32 k-tiles, Scalar abs -> tiny PE matmuls against a ones vector
    accumulate colsum|w| in PSUM [1, 128]; ws = colsum/DI + eps;
    rw = 1/ws broadcast to all partitions (gpsimd partition_broadcast);
    quantize each k-tile to ternary bf16 via magic-number round + clamp.
    ws*alpha bounced through a DRAM scratch row into per-o-tile columns
    for the epilogue.
  A phase (16 token-chunks of 256): stream xT chunk as 32 k-tiles
    [128, 256] f32, Scalar abs + DVE running max -> m [128, 256];
    cross-partition max via gpsimd partition_all_reduce -> amax
    (already replicated on all partitions); scl = 127/(amax+eps);
    stok = (amax+eps)/127; quantize k-tiles to integer bf16 (DVE mult,
    Scalar magic round).
  Matmul: yT accumulation, weights stationary: for each o-tile j (8),
    32 matmuls psum[128 o, 256 t] += wq[kk, j].T @ xq[kk].
  Epilogue: Scalar copies PSUM -> SBUF scaled by ws*alpha (per
    partition = per out feature), DVE multiplies by the replicated
    stok row (per token), DMA out to yT.

The quantized operands are exact in bf16 (integers <= 127, ternary
weights), so the bf16 matmul is bit-exact integer arithmetic.
"""
import sys

sys.path.insert(0, "/opt/trn_rl_repo")

import numpy as np

B, S, DI, DOUT = 4, 2048, 4096, 4096
DP, TP = 2, 4
T_C = B * S // DP      # 4096 tokens per core
O_C = DOUT // TP       # 1024 out features per core
NK = DI // 128         # 32 k-tiles
TCW = 256              # token chunk width
NTC = T_C // TCW       # 16 token chunks
OCW = 128              # out-feature chunk width (one o-tile)
NOC = O_C // OCW       # 8 o-chunks == o-tiles

EPS = 1e-8
QMAX = 127.0
MAGIC = 12582912.0     # 1.5 * 2**23; f32 add/sub rounds to nearest-even int

_cached = {}


def _install_walrus_workarounds(tile_mod, mybir):
    """This walrus build rejects instructions with more than one sem wait
    ('Too many sync wait commands'). Split the Tile tail-drain waits over
    several sequencer drains; regular instructions are handled by
    _split_sync_waits after scheduling."""
    from concourse.vector_clock import ScopedClock

    def _drain_and_barrier_split(self, tick_clock, wait_clock):
        drain_inst = self.nc.sync.drain()
        wait_clock.add_sem_waits(
            drain_inst.ins, ScopedClock({None: tick_clock.global_clock})
        )
        waits = list(drain_inst.ins.sync_info.on_wait)
        if len(waits) > 1:
            del drain_inst.ins.sync_info.on_wait[1:]
            for w in waits[1:]:
                extra = self.nc.sync.drain()
                extra.ins.sync_info = mybir.SyncInfo(on_wait=[w], on_update=[])

        self.nc.all_engine_barrier()
        assert self.sems is not None
        popped = self.nc._tile_sem_poison_stack.pop()
        assert popped is self._sem_poison
        self.nc.clear_and_free_semaphores(list(self.sems.allocated().values()))
        self.nc.all_engine_barrier()

    tile_mod.TileContext._drain_and_barrier = _drain_and_barrier_split


def _split_sync_waits(nc, mybir, max_waits=1):
    """Move excess sem waits onto same-engine NoOps inserted before the
    offending instruction (engines run their stream in order, so the wait
    conjunction is preserved)."""
    n = 0
    for fn in nc.m.functions:
        for bb in fn.blocks:
            insts = bb.instructions
            i = 0
            while i < len(insts):
                inst = insts[i]
                si = getattr(inst, "sync_info", None)
                if si is not None and si.on_wait and len(si.on_wait) > max_waits:
                    waits = list(si.on_wait)
                    extra = waits[: len(waits) - max_waits]
                    del si.on_wait[: len(waits) - max_waits]
                    nops = []
                    for j in range(0, len(extra), max_waits):
                        nop = mybir.InstNoOp(name=f"WSPLIT-{n}", ins=[], outs=[])
                        n += 1
                        nop.engine = inst.engine
                        nop.sync_info = mybir.SyncInfo(
                            on_wait=list(extra[j : j + max_waits]), on_update=[]
                        )
                        nops.append(nop)
                    insts[i:i] = nops
                    i += len(nops)
                i += 1
    return n


def _build():
    import contextlib

    import concourse.bass as bass
    import concourse.tile as tile
    from concourse import masks, mybir

    _install_walrus_workarounds(tile, mybir)

    F32 = mybir.dt.float32
    BF16 = mybir.dt.bfloat16
    Alu = mybir.AluOpType
    Act = mybir.ActivationFunctionType
    Ax = mybir.AxisListType

    nc = bass.Bass("TRN2", target_bir_lowering=False, debug=False, num_devices=8)
    x_d = nc.declare_dram_parameter("x", [DI, T_C], F32, isOutput=False)
    w_d = nc.declare_dram_parameter("w", [DI, O_C], F32, isOutput=False)
    al_d = nc.declare_dram_parameter("alpha", [O_C], F32, isOutput=False)
    y_d = nc.declare_dram_parameter("y", [O_C, T_C], F32, isOutput=True)
    scr_d = nc.dram_tensor("wsa_scratch", [O_C], F32)

    with tile.TileContext(nc) as tc, contextlib.ExitStack() as ctx:
        wst = ctx.enter_context(tc.tile_pool(name="wst", bufs=64))
        xst = ctx.enter_context(tc.tile_pool(name="xst", bufs=39))
        xqp = ctx.enter_context(tc.tile_pool(name="xqp", bufs=2))
        wqp = ctx.enter_context(tc.tile_pool(name="wqp", bufs=1))
        wab = ctx.enter_context(tc.tile_pool(name="wab", bufs=3))
        tmw = ctx.enter_context(tc.tile_pool(name="tmw", bufs=4))
        tmx = ctx.enter_context(tc.tile_pool(name="tmx", bufs=3))
        mst = ctx.enter_context(tc.tile_pool(name="mst", bufs=2))
        stt = ctx.enter_context(tc.tile_pool(name="stt", bufs=2))
        row = ctx.enter_context(tc.tile_pool(name="row", bufs=2))
        bcr = ctx.enter_context(tc.tile_pool(name="bcr", bufs=2))
        sbp = ctx.enter_context(tc.tile_pool(name="sbp", bufs=3))
        cst = ctx.enter_context(tc.tile_pool(name="cst", bufs=1))
        ps = ctx.enter_context(tc.tile_pool(name="ps", bufs=4, space="PSUM"))
        psw = ctx.enter_context(tc.tile_pool(name="psw", bufs=1, space="PSUM"))
        psS = ctx.enter_context(tc.tile_pool(name="psS", bufs=1, space="PSUM"))

        ones = cst.tile([128, 1], BF16, tag="ones")
        nc.vector.memset(ones, 1.0)
        # row of f32 ones: K=1 matmul against it broadcasts a [1, N] row
        # to all 128 output partitions
        ones_row = cst.tile([1, 128], F32, tag="onesrow")
        nc.vector.memset(ones_row, 1.0)
        ident = cst.tile([128, 128], F32, tag="ident")
        masks.make_identity(nc, ident[:, :])
        posmagic = cst.tile([128, 1], F32, tag="posmagic")
        nc.vector.memset(posmagic, MAGIC)
        negmagic = cst.tile([128, 1], F32, tag="negmagic")
        nc.vector.memset(negmagic, -MAGIC)
        # ws*alpha columns, one per o-tile, written via DRAM scratch bounce
        wsa_cols = cst.tile([128, NOC], F32, tag="wsacols", name="wsacols")
        # ternary weights, k on partitions: [128, kk, o]
        wq = wqp.tile([128, NK, O_C], BF16, tag="wq", name="wq")

        def emit_w(c):
            o0 = c * OCW
            parts = []
            for kk in range(NK):
                wt = wst.tile([128, OCW], F32, tag="wld")
                nc.sync.dma_start(
                    out=wt, in_=w_d[kk * 128:(kk + 1) * 128, o0:o0 + OCW]
                )
                parts.append(wt)
            wsum = psw.tile([1, OCW], F32, tag="wsum", name=f"wsum{c}")
            for kk in range(NK):
                ab = wab.tile([128, OCW], BF16, tag="wabs")
                nc.scalar.activation(out=ab, in_=parts[kk], func=Act.Abs)
                nc.tensor.matmul(
                    wsum, lhsT=ones, rhs=ab,
                    start=(kk == 0), stop=(kk == NK - 1),
                )
            ws_row = row.tile([1, OCW], F32, tag="wsrow")
            nc.vector.tensor_scalar(
                out=ws_row, in0=wsum, scalar1=1.0 / DI, scalar2=EPS,
                op0=Alu.mult, op1=Alu.add,
            )
            rw_row = row.tile([1, OCW], F32, tag="rwrow")
            nc.vector.reciprocal(out=rw_row, in_=ws_row)
            al_row = row.tile([1, OCW], F32, tag="alrow")
            nc.gpsimd.dma_start(
                out=al_row,
                in_=al_d[o0:o0 + OCW].rearrange("(u o) -> u o", u=1),
            )
            wsa_row = row.tile([1, OCW], F32, tag="wsarow")
            nc.vector.tensor_tensor(
                out=wsa_row, in0=ws_row, in1=al_row, op=Alu.mult
            )
            nc.gpsimd.dma_start(
                out=bass.AP(tensor=scr_d, offset=o0, ap=[[1, OCW]]),
                in_=wsa_row,
            )
            nc.gpsimd.dma_start(
                out=wsa_cols[:, c:c + 1],
                in_=bass.AP(tensor=scr_d, offset=o0, ap=[[1, OCW]]),
            )
            bcw = psS.tile([128, TCW], F32, tag="bc", name=f"bcw{c}")
            nc.tensor.matmul(
                bcw[:, :OCW], lhsT=ones_row, rhs=rw_row, start=True, stop=True
            )
            rw_bc = bcr.tile([128, OCW], F32, tag="rwbc")
            nc.scalar.activation(out=rw_bc, in_=bcw[:, :OCW], func=Act.Copy)
            for kk in range(NK):
                wn = tmw.tile([128, OCW], F32, tag="wn")
                nc.vector.tensor_tensor(
                    out=wn, in0=parts[kk], in1=rw_bc, op=Alu.mult
                )
                m1 = tmw.tile([128, OCW], F32, tag="wm1")
                nc.scalar.activation(
                    out=m1, in_=wn, func=Act.Identity, bias=posmagic
                )
                m2 = tmw.tile([128, OCW], F32, tag="wm2")
                nc.vector.tensor_scalar(
                    out=m2, in0=m1, scalar1=-MAGIC, scalar2=1.0,
                    op0=Alu.add, op1=Alu.min,
                )
                nc.vector.tensor_scalar_max(
                    out=wq[:, kk, o0:o0 + OCW], in0=m2, scalar1=-1.0
                )

        def emit_a(t):
            t0 = t * TCW
            parts = []
            for kk in range(NK):
                xt = xst.tile([128, TCW], F32, tag="xld")
                nc.sync.dma_start(
                    out=xt, in_=x_d[kk * 128:(kk + 1) * 128, t0:t0 + TCW]
                )
                parts.append(xt)
            m = mst.tile([128, TCW], F32, tag="m")
            nc.scalar.activation(out=m, in_=parts[0], func=Act.Abs)
            for kk in range(1, NK):
                ax = tmx.tile([128, TCW], F32, tag="ax")
                nc.scalar.activation(out=ax, in_=parts[kk], func=Act.Abs)
                nc.vector.tensor_tensor(out=m, in0=m, in1=ax, op=Alu.max)
            # cross-partition max: PE-transpose each 128-token half of m,
            # DVE-reduce along the (now free) k-group axis, add eps, then
            # PE-transpose the per-token column back into a row
            teps_row = row.tile([1, TCW], F32, tag="tepsrow")
            for h in range(2):
                mt = psS.tile([128, 128], F32, tag="mt", name=f"mt{t}_{h}")
                nc.tensor.transpose(mt, m[:, h * 128:(h + 1) * 128], ident)
                acol = stt.tile([128, 1], F32, tag="acol")
                nc.vector.tensor_reduce(
                    out=acol, in_=mt, axis=Ax.X, op=Alu.max
                )
                tcol = stt.tile([128, 1], F32, tag="tcol")
                nc.vector.tensor_scalar_add(out=tcol, in0=acol, scalar1=EPS)
                trow = psS.tile([1, 128], F32, tag="trow", name=f"tr{t}_{h}")
                nc.tensor.transpose(trow, tcol, ident)
                nc.vector.tensor_copy(
                    out=teps_row[:, h * 128:(h + 1) * 128], in_=trow
                )
            rrow = row.tile([1, TCW], F32, tag="rrow")
            nc.vector.reciprocal(out=rrow, in_=teps_row)
            scl_row = row.tile([1, TCW], F32, tag="sclrow")
            nc.vector.tensor_scalar_mul(out=scl_row, in0=rrow, scalar1=QMAX)
            bc = psS.tile([128, TCW], F32, tag="bc", name=f"bc{t}")
            nc.tensor.matmul(bc, lhsT=ones_row, rhs=scl_row, start=True, stop=True)
            scl = stt.tile([128, TCW], F32, tag="scl")
            nc.scalar.activation(out=scl, in_=bc, func=Act.Copy)
            stok = stt.tile([128, TCW], F32, tag="stok")
            nc.vector.reciprocal(out=stok, in_=scl)
            xq = xqp.tile([128, NK, TCW], BF16, tag="xq", name=f"xq{t}")
            for kk in range(NK):
                xm = tmx.tile([128, TCW], F32, tag="xm")
                nc.vector.tensor_tensor(
                    out=xm, in0=parts[kk], in1=scl, op=Alu.mult
                )
                q1 = tmx.tile([128, TCW], F32, tag="q1")
                nc.scalar.activation(
                    out=q1, in_=xm, func=Act.Identity, bias=posmagic
                )
                nc.scalar.activation(
                    out=xq[:, kk, :], in_=q1, func=Act.Identity, bias=negmagic
                )
            return xq, stok

        def emit_mm_epi(t, j, xq, stok):
            t0 = t * TCW
            psum = ps.tile([128, TCW], F32, tag="psum", name=f"ps{t}_{j}")
            for kk in range(NK):
                nc.tensor.matmul(
                    psum,
                    lhsT=wq[:, kk, j * OCW:(j + 1) * OCW],
                    rhs=xq[:, kk, :],
                    start=(kk == 0),
                    stop=(kk == NK - 1),
                )
            sb = sbp.tile([128, TCW], F32, tag="sb")
            nc.scalar.activation(
                out=sb, in_=psum, func=Act.Copy, bias=0.0,
                scale=wsa_cols[:, j:j + 1],
            )
            nc.vector.tensor_tensor(out=sb, in0=sb, in1=stok, op=Alu.mult)
            nc.gpsimd.dma_start(
                out=y_d[j * OCW:(j + 1) * OCW, t0:t0 + TCW], in_=sb
            )

        # Emission order doubles as scheduling priority AND correctness
        # (Tile's dependency tracking is history-based: readers must be
        # emitted after the writer). W o-chunk c fills wq[:, :, c*128:...]
        # and wsa_cols[:, c]; tc0's matmul for o-tile j is emitted right
        # after W chunk j+1 so the PE starts early and chunk loads/quant
        # stay ahead of the matmul chase.
        emit_w(0)
        emit_w(1)
        A = [None] * NTC
        A[0] = emit_a(0)
        emit_mm_epi(0, 0, *A[0])
        for c in range(2, NOC):
            emit_w(c)
            emit_mm_epi(0, c - 1, *A[0])
        A[1] = emit_a(1)
        emit_mm_epi(0, NOC - 1, *A[0])
        for t in range(1, NTC):
            for j in range(NOC):
                if j == 3 and t + 1 < NTC:
                    A[t + 1] = emit_a(t + 1)
                emit_mm_epi(t, j, *A[t])

    _split_sync_waits(nc, mybir, max_waits=1)
    return nc


def _get_nc():
    if "nc" not in _cached:
        _cached["nc"] = _build()
    return _cached["nc"]


def _run(x, weight, alpha, trace=False):
    from concourse.bass_utils import run_bass_kernel_spmd

    nc = _get_nc()
    x_flat = np.asarray(x).reshape(B * S, DI)
    xT = np.ascontiguousarray(x_flat.T)          # [DI, B*S]
    weight = np.asarray(weight)
    alpha = np.asarray(alpha)
    in_maps = []
    for c in range(8):
        dp, tp = divmod(c, TP)
        in_maps.append(
            {
                "x": np.ascontiguousarray(xT[:, dp * T_C:(dp + 1) * T_C]),
                "w": np.ascontiguousarray(
                    weight[tp * O_C:(tp + 1) * O_C, :].T
                ),
                "alpha": np.ascontiguousarray(alpha[tp * O_C:(tp + 1) * O_C]),
            }
        )
    res = run_bass_kernel_spmd(nc, in_maps, list(range(8)), trace=trace)
    y = np.empty((B * S, DOUT), np.float32)
    for c in range(8):
        dp, tp = divmod(c, TP)
        y[dp * T_C:(dp + 1) * T_C, tp * O_C:(tp + 1) * O_C] = (
            res.results[c]["y"].T
        )
    return y.reshape(B, S, DOUT), res


def kernel(x, weight, alpha):
    y, _ = _run(x, weight, alpha, trace=False)
    return y


# revision 18
# speedup vs baseline: 1.2966x; 1.0978x over previous
"""BitLinear forward on 8 Trainium2 NeuronCores.

Sharding: 2-way data parallel over tokens x 4-way tensor parallel over
output features. Each core computes yT[out_quarter, token_half] =
[1024, 4096] of the full output; the host transposes/assembles.

Layout strategy: the host passes x and w PRE-TRANSPOSED (contraction
dim k major): xT [4096 k, 4096 t] and wT [4096 k, 1024 o] per core.
All DMAs are then contiguous row chunks (1-2 KiB per partition line)
and the kernel needs NO bulk on-chip transposes (the old kernel's
dma_start_transpose descriptor storm was the main bottleneck).

Per-core pipeline:
  W phase (8 o-chunks of 128): stream wT chunk [4096 k, 128 o] f32 as
    32 k-tiles, Scalar abs -> tiny PE matmuls against a ones vector
    accumulate colsum|w| in PSUM [1, 128]; ws = colsum/DI + eps;
    rw = 1/ws broadcast to all partitions (gpsimd partition_broadcast);
    quantize each k-tile to ternary bf16 via magic-number round + clamp.
    ws*alpha bounced through a DRAM scratch row into per-o-tile columns
    for the epilogue.
  A phase (16 token-chunks of 256): stream xT chunk as 32 k-tiles
    [128, 256] f32, Scalar abs + DVE running max -> m [128, 256];
    cross-partition max via gpsimd partition_all_reduce -> amax
    (already replicated on all partitions); scl = 127/(amax+eps);
    stok = (amax+eps)/127; quantize k-tiles to integer bf16 (DVE mult,
    Scalar magic round).
  Matmul: yT accumulation, weights stationary: for each o-tile j (8),
    32 matmuls psum[128 o, 256 t] += wq[kk, j].T @ xq[kk].
  Epilogue: Scalar copies PSUM -> SBUF scaled by ws*alpha (per
    partition = per out feature), DVE multiplies by the replicated
    stok row (per token), DMA out to yT.

The quantized operands are exact in bf16 (integers <= 127, ternary
weights), so the bf16 matmul is bit-exact integer arithmetic.
"""
import sys

sys.path.insert(0, "/opt/trn_rl_repo")

import numpy as np

B, S, DI, DOUT = 4, 2048, 4096, 4096
DP, TP = 2, 4
T_C = B * S // DP      # 4096 tokens per core
O_C = DOUT // TP       # 1024 out features per core
NK = DI // 128         # 32 k-tiles
TCW = 256              # token chunk width
NTC = T_C // TCW       # 16 token chunks
OCW = 128              # out-feature chunk width (one o-tile)
NOC = O_C // OCW       # 8 o-chunks == o-tiles

EPS = 1e-8
QMAX = 127.0
MAGIC = 12582912.0     # 1.5 * 2**23; f32 add/sub rounds to nearest-even int

_cached = {}


def _install_walrus_workarounds(tile_mod, mybir):
    """This walrus build rejects instructions with more than one sem wait
    ('Too many sync wait commands'). Split the Tile tail-drain waits over
    several sequencer drains; regular instructions are handled by
    _split_sync_waits after scheduling."""
    from concourse.vector_clock import ScopedClock

    def _drain_and_barrier_split(self, tick_clock, wait_clock):
        drain_inst = self.nc.sync.drain()
        wait_clock.add_sem_waits(
            drain_inst.ins, ScopedClock({None: tick_clock.global_clock})
        )
        waits = list(drain_inst.ins.sync_info.on_wait)
        if len(waits) > 1:
            del drain_inst.ins.sync_info.on_wait[1:]
            for w in waits[1:]:
                extra = self.nc.sync.drain()
                extra.ins.sync_info = mybir.SyncInfo(on_wait=[w], on_update=[])

        self.nc.all_engine_barrier()
        assert self.sems is not None
        popped = self.nc._tile_sem_poison_stack.pop()
        assert popped is self._sem_poison
        self.nc.clear_and_free_semaphores(list(self.sems.allocated().values()))
        self.nc.all_engine_barrier()

    tile_mod.TileContext._drain_and_barrier = _drain_and_barrier_split


def _split_sync_waits(nc, mybir, max_waits=1):
    """Move excess sem waits onto same-engine NoOps inserted before the
    offending instruction (engines run their stream in order, so the wait
    conjunction is preserved)."""
    n = 0
    for fn in nc.m.functions:
        for bb in fn.blocks:
            insts = bb.instructions
            i = 0
            while i < len(insts):
                inst = insts[i]
                si = getattr(inst, "sync_info", None)
                if si is not None and si.on_wait and len(si.on_wait) > max_waits:
                    waits = list(si.on_wait)
                    extra = waits[: len(waits) - max_waits]
                    del si.on_wait[: len(waits) - max_waits]
                    nops = []
                    for j in range(0, len(extra), max_waits):
                        nop = mybir.InstNoOp(name=f"WSPLIT-{n}", ins=[], outs=[])
                        n += 1
                        nop.engine = inst.engine
                        nop.sync_info = mybir.SyncInfo(
                            on_wait=list(extra[j : j + max_waits]), on_update=[]
                        )
                        nops.append(nop)
                    insts[i:i] = nops
                    i += len(nops)
                i += 1
    return n


def _build():
    import contextlib

    import concourse.bass as bass
    import concourse.tile as tile
    from concourse import masks, mybir

    _install_walrus_workarounds(tile, mybir)

    F32 = mybir.dt.float32
    BF16 = mybir.dt.bfloat16
    Alu = mybir.AluOpType
    Act = mybir.ActivationFunctionType
    Ax = mybir.AxisListType

    nc = bass.Bass("TRN2", target_bir_lowering=False, debug=False, num_devices=8)
    x_d = nc.declare_dram_parameter("x", [DI, T_C], F32, isOutput=False)
    w_d = nc.declare_dram_parameter("w", [DI, O_C], F32, isOutput=False)
    al_d = nc.declare_dram_parameter("alpha", [O_C], F32, isOutput=False)
    y_d = nc.declare_dram_parameter("y", [O_C, T_C], F32, isOutput=True)
    scr_d = nc.dram_tensor("wsa_scratch", [O_C], F32)

    with tile.TileContext(nc) as tc, contextlib.ExitStack() as ctx:
        wst = ctx.enter_context(tc.tile_pool(name="wst", bufs=10))
        xst = ctx.enter_context(tc.tile_pool(name="xst", bufs=10))
        xqp = ctx.enter_context(tc.tile_pool(name="xqp", bufs=2))
        wqp = ctx.enter_context(tc.tile_pool(name="wqp", bufs=1))
        wab = ctx.enter_context(tc.tile_pool(name="wab", bufs=3))
        tmx = ctx.enter_context(tc.tile_pool(name="tmx", bufs=2))
        mst = ctx.enter_context(tc.tile_pool(name="mst", bufs=2))
        stt = ctx.enter_context(tc.tile_pool(name="stt", bufs=2))
        row = ctx.enter_context(tc.tile_pool(name="row", bufs=2))
        bcr = ctx.enter_context(tc.tile_pool(name="bcr", bufs=2))
        sbp = ctx.enter_context(tc.tile_pool(name="sbp", bufs=3))
        cst = ctx.enter_context(tc.tile_pool(name="cst", bufs=1))
        ps = ctx.enter_context(tc.tile_pool(name="ps", bufs=4, space="PSUM"))
        psw = ctx.enter_context(tc.tile_pool(name="psw", bufs=1, space="PSUM"))
        psS = ctx.enter_context(tc.tile_pool(name="psS", bufs=1, space="PSUM"))

        ones = cst.tile([128, 1], BF16, tag="ones")
        nc.vector.memset(ones, 1.0)
        # row of f32 ones: K=1 matmul against it broadcasts a [1, N] row
        # to all 128 output partitions
        ones_row = cst.tile([1, 128], F32, tag="onesrow")
        nc.vector.memset(ones_row, 1.0)
        ident = cst.tile([128, 128], F32, tag="ident")
        masks.make_identity(nc, ident[:, :])
        posmagic = cst.tile([128, 1], F32, tag="posmagic")
        nc.vector.memset(posmagic, MAGIC)
        negmagic = cst.tile([128, 1], F32, tag="negmagic")
        nc.vector.memset(negmagic, -MAGIC)
        # ws*alpha columns, one per o-tile, written via DRAM scratch bounce
        wsa_cols = cst.tile([128, NOC], F32, tag="wsacols", name="wsacols")
        # ternary weights, k on partitions: [128, kk, o]
        wq = wqp.tile([128, NK, O_C], BF16, tag="wq", name="wq")

        def emit_w(c):
            o0 = c * OCW
            quads = []
            for q in range(NK // 4):
                wt = wst.tile([128, 4, OCW], F32, tag="wld")
                nc.sync.dma_start(
                    out=wt,
                    in_=w_d[512 * q:512 * (q + 1), o0:o0 + OCW].rearrange(
                        "(i p) o -> p i o", p=128
                    ),
                )
                quads.append(wt)
            wsum = psw.tile([1, 4, OCW], F32, tag="wsum", name=f"wsum{c}")
            for q in range(NK // 4):
                ab = wab.tile([128, 4, OCW], BF16, tag="wabs")
                nc.scalar.activation(out=ab, in_=quads[q], func=Act.Abs)
                nc.tensor.matmul(
                    wsum, lhsT=ones, rhs=ab,
                    start=(q == 0), stop=(q == NK // 4 - 1),
                )
            wsum_sb = row.tile([1, 4, OCW], F32, tag="wsums")
            nc.scalar.activation(out=wsum_sb, in_=wsum, func=Act.Copy)
            ws_row = row.tile([1, OCW], F32, tag="wsrow")
            nc.vector.tensor_tensor(
                out=ws_row, in0=wsum_sb[:, 0, :], in1=wsum_sb[:, 1, :],
                op=Alu.add,
            )
            wt2 = row.tile([1, OCW], F32, tag="wtmp")
            nc.vector.tensor_tensor(
                out=wt2, in0=wsum_sb[:, 2, :], in1=wsum_sb[:, 3, :],
                op=Alu.add,
            )
            nc.vector.tensor_tensor(
                out=ws_row, in0=ws_row, in1=wt2, op=Alu.add
            )
            nc.vector.tensor_scalar(
                out=ws_row, in0=ws_row, scalar1=1.0 / DI, scalar2=EPS,
                op0=Alu.mult, op1=Alu.add,
            )
            rw_row = row.tile([1, OCW], F32, tag="rwrow")
            nc.vector.reciprocal(out=rw_row, in_=ws_row)
            al_row = row.tile([1, OCW], F32, tag="alrow")
            nc.gpsimd.dma_start(
                out=al_row,
                in_=al_d[o0:o0 + OCW].rearrange("(u o) -> u o", u=1),
            )
            wsa_row = row.tile([1, OCW], F32, tag="wsarow")
            nc.vector.tensor_tensor(
                out=wsa_row, in0=ws_row, in1=al_row, op=Alu.mult
            )
            nc.gpsimd.dma_start(
                out=bass.AP(tensor=scr_d, offset=o0, ap=[[1, OCW]]),
                in_=wsa_row,
            )
            nc.gpsimd.dma_start(
                out=wsa_cols[:, c:c + 1],
                in_=bass.AP(tensor=scr_d, offset=o0, ap=[[1, OCW]]),
            )
            bcw = psS.tile([128, TCW], F32, tag="bc", name=f"bcw{c}")
            nc.tensor.matmul(
                bcw[:, :OCW], lhsT=ones_row, rhs=rw_row, start=True, stop=True
            )
            rw_big = bcr.tile([128, 4, OCW], F32, tag="rwbig")
            for i in range(4):
                nc.scalar.activation(
                    out=rw_big[:, i, :], in_=bcw[:, :OCW], func=Act.Copy
                )
            for q in range(NK // 4):
                nc.vector.tensor_tensor(
                    out=quads[q], in0=quads[q], in1=rw_big, op=Alu.mult
                )
                nc.vector.tensor_scalar(
                    out=quads[q], in0=quads[q], scalar1=1.0, scalar2=-1.0,
                    op0=Alu.min, op1=Alu.max,
                )
                nc.scalar.activation(
                    out=quads[q], in_=quads[q], func=Act.Identity,
                    bias=posmagic,
                )
                nc.scalar.activation(
                    out=wq[:, 4 * q:4 * q + 4, o0:o0 + OCW], in_=quads[q],
                    func=Act.Identity, bias=negmagic,
                )

        def emit_a(t):
            t0 = t * TCW
            quads = []
            for q in range(NK // 4):
                xt = xst.tile([128, 4, TCW], F32, tag="xld")
                nc.sync.dma_start(
                    out=xt,
                    in_=x_d[512 * q:512 * (q + 1), t0:t0 + TCW].rearrange(
                        "(i p) t -> p i t", p=128
                    ),
                )
                quads.append(xt)
            acc = mst.tile([128, 4, TCW], F32, tag="acc")
            nc.scalar.activation(out=acc, in_=quads[0], func=Act.Abs)
            for q in range(1, NK // 4):
                ax = tmx.tile([128, 4, TCW], F32, tag="ax")
                nc.scalar.activation(out=ax, in_=quads[q], func=Act.Abs)
                nc.vector.tensor_tensor(
                    out=acc, in0=acc, in1=ax, op=Alu.max
                )
            m = mst.tile([128, TCW], F32, tag="m")
            nc.vector.tensor_tensor(
                out=m, in0=acc[:, 0, :], in1=acc[:, 1, :], op=Alu.max
            )
            nc.vector.tensor_tensor(
                out=m, in0=m, in1=acc[:, 2, :], op=Alu.max
            )
            nc.vector.tensor_tensor(
                out=m, in0=m, in1=acc[:, 3, :], op=Alu.max
            )
            # cross-partition max: PE-transpose each 128-token half of m,
            # DVE-reduce along the (now free) k-group axis, add eps, then
            # PE-transpose the per-token column back into a row
            teps_row = row.tile([1, TCW], F32, tag="tepsrow")
            for h in range(2):
                mt = psS.tile([128, 128], F32, tag="mt", name=f"mt{t}_{h}")
                nc.tensor.transpose(mt, m[:, h * 128:(h + 1) * 128], ident)
                acol = stt.tile([128, 1], F32, tag="acol")
                nc.vector.tensor_reduce(
                    out=acol, in_=mt, axis=Ax.X, op=Alu.max
                )
                nc.vector.tensor_scalar_add(out=acol, in0=acol, scalar1=EPS)
                trow = psS.tile([1, 128], F32, tag="trow", name=f"tr{t}_{h}")
                nc.tensor.transpose(trow, acol, ident)
                nc.vector.tensor_copy(
                    out=teps_row[:, h * 128:(h + 1) * 128], in_=trow
                )
            nc.vector.reciprocal(out=teps_row, in_=teps_row)
            nc.vector.tensor_scalar_mul(
                out=teps_row, in0=teps_row, scalar1=QMAX
            )
            bc = psS.tile([128, TCW], F32, tag="bc", name=f"bc{t}")
            nc.tensor.matmul(
                bc, lhsT=ones_row, rhs=teps_row, start=True, stop=True
            )
            scl_big = stt.tile([128, 4, TCW], F32, tag="sclbig")
            for i in range(4):
                nc.scalar.activation(
                    out=scl_big[:, i, :], in_=bc, func=Act.Copy
                )
            stok = stt.tile([128, TCW], F32, tag="stok")
            nc.vector.reciprocal(out=stok, in_=bc)
            xq = xqp.tile([128, NK, TCW], BF16, tag="xq", name=f"xq{t}")
            for q in range(NK // 4):
                nc.vector.tensor_tensor(
                    out=quads[q], in0=quads[q], in1=scl_big, op=Alu.mult
                )
                nc.vector.tensor_scalar(
                    out=xq[:, 4 * q:4 * q + 4, :], in0=quads[q],
                    scalar1=MAGIC, scalar2=MAGIC,
                    op0=Alu.add, op1=Alu.subtract,
                )
            return xq, stok

        def emit_mm_epi(t, j, xq, stok):
            t0 = t * TCW
            psum = ps.tile([128, TCW], F32, tag="psum", name=f"ps{t}_{j}")
            for kk in range(NK):
                nc.tensor.matmul(
                    psum,
                    lhsT=wq[:, kk, j * OCW:(j + 1) * OCW],
                    rhs=xq[:, kk, :],
                    start=(kk == 0),
                    stop=(kk == NK - 1),
                )
            sb = sbp.tile([128, TCW], F32, tag="sb")
            nc.scalar.activation(
                out=sb, in_=psum, func=Act.Copy, bias=0.0,
                scale=wsa_cols[:, j:j + 1],
            )
            nc.vector.tensor_tensor(out=sb, in0=sb, in1=stok, op=Alu.mult)
            nc.gpsimd.dma_start(
                out=y_d[j * OCW:(j + 1) * OCW, t0:t0 + TCW], in_=sb
            )

        # Emission order doubles as scheduling priority AND correctness
        # (Tile's dependency tracking is history-based: readers must be
        # emitted after the writer). W o-chunk c fills wq[:, :, c*128:...]
        # and wsa_cols[:, c]; tc0's matmul for o-tile j is emitted right
        # after W chunk j+1 so the PE starts early and chunk loads/quant
        # stay ahead of the matmul chase.
        emit_w(0)
        emit_w(1)
        A = [None] * NTC
        A[0] = emit_a(0)
        emit_mm_epi(0, 0, *A[0])
        for c in range(2, NOC):
            emit_w(c)
            emit_mm_epi(0, c - 1, *A[0])
        A[1] = emit_a(1)
        emit_mm_epi(0, NOC - 1, *A[0])
        for t in range(1, NTC):
            for j in range(NOC):
                if j == 3 and t + 1 < NTC:
                    A[t + 1] = emit_a(t + 1)
                emit_mm_epi(t, j, *A[t])

    _split_sync_waits(nc, mybir, max_waits=1)
    return nc


def _get_nc():
    if "nc" not in _cached:
        _cached["nc"] = _build()
    return _cached["nc"]


def _run(x, weight, alpha, trace=False):
    from concourse.bass_utils import run_bass_kernel_spmd

    nc = _get_nc()
    x_flat = np.asarray(x).reshape(B * S, DI)
    xT = np.ascontiguousarray(x_flat.T)          # [DI, B*S]
    weight = np.asarray(weight)
    alpha = np.asarray(alpha)
    in_maps = []
    for c in range(8):
        dp, tp = divmod(c, TP)
        in_maps.append(
            {
                "x": np.ascontiguousarray(xT[:, dp * T_C:(dp + 1) * T_C]),
                "w": np.ascontiguousarray(
                    weight[tp * O_C:(tp + 1) * O_C, :].T
                ),
                "alpha": np.ascontiguousarray(alpha[tp * O_C:(tp + 1) * O_C]),
            }
        )
    res = run_bass_kernel_spmd(nc, in_maps, list(range(8)), trace=trace)
    y = np.empty((B * S, DOUT), np.float32)
    for c in range(8):
        dp, tp = divmod(c, TP)
        y[dp * T_C:(dp + 1) * T_C, tp * O_C:(tp + 1) * O_C] = (
            res.results[c]["y"].T
        )
    return y.reshape(B, S, DOUT), res


def kernel(x, weight, alpha):
    y, _ = _run(x, weight, alpha, trace=False)
    return y
